# revision 6
# baseline (speedup 1.0000x reference)
"""ALiBi multi-head attention (B=4, Tq=1024, D=1024, H=16, cache=1024) on 8
Trainium2 NeuronCores.

Sharding: core c = (batch b = c//2, head-group g = c%2). Each core runs one
batch with 8 of the 16 heads. Heads are assigned to groups interleaved by
ALiBi window size so per-core work balances, and all cores run an identical
SPMD graph (per-slot key windows are the max over the two groups).

Device dataflow (all transposed so ALiBi becomes a per-partition ACT bias):
  qT = Wq_s^T x^T + bq        kTnew = Wk_s^T x^T        vnew = x Wv_s + bv
  scoresT[k,q] = kT_chunk^T qT        (K=64 contraction, 2-head row packing)
  P^T = exp(0.125*scoresT + alibi[k])  (single ScalarE pass, PSUM->SBUF bf16)
  oT += v_chunk^T P^T  (V stationary, col-packed pairs -> PSUM[128,1024])
  denom += ones^T P^T  (M=64 ones lhsT -> denominator broadcast to 64 rows)
  wvT = oT * recip(denom);  outT_partial = Wo_s^T wvT + 0.5*bo
Softmax max-subtraction is skipped (scores are O(1), bias <= 0) and keys with
alibi bias < -16 are dropped (error ~1e-7 << tolerance).

Host: shards/pre-transposes inputs (bf16), sums the two partial outT per
batch, scatters head-sliced k/v outputs. No collectives on device.
"""

import numpy as np
import ml_dtypes

BF16 = ml_dtypes.bfloat16

B, Tq, D = 4, 1024, 1024
H, DH = 16, 64
CACHE = 1024
Tk = CACHE + Tq
NCH = Tk // 128  # 16 key chunks
T_CUT = 16.0

# ---- head assignment / windows (hardcoded, deterministic) ----
_slopes = 2.0 ** (-(8.0 / H) * np.arange(1, H + 1))  # head h=0..15 -> slope
_raw = np.minimum(np.ceil(T_CUT / _slopes), Tk)
_W = np.minimum(((_raw + 127) // 128) * 128, Tk).astype(int)
_order = np.argsort(-_W, kind="stable")
SLOT_W = [int(max(_W[_order[2 * s]], _W[_order[2 * s + 1]])) for s in range(8)]
HEADS_OF_GROUP = [[int(_order[2 * s + g]) for s in range(8)] for g in (0, 1)]
COLS_G = [
    np.concatenate([np.arange(h * DH, (h + 1) * DH) for h in HEADS_OF_GROUP[g]])
    for g in (0, 1)
]

_COMPILED = None


def _build():
    import concourse.bacc as bacc
    import concourse.tile as tile
    import concourse.mybir as mybir

    f32 = mybir.dt.float32
    bf16 = mybir.dt.bfloat16
    ADD = mybir.AluOpType.add
    MUL = mybir.AluOpType.mult
    EXP = mybir.ActivationFunctionType.Exp

    nc = bacc.Bacc("TRN2", target_bir_lowering=False, debug=False)

    xt = nc.dram_tensor("xt", [D, Tq], bf16, kind="ExternalInput")
    wq = nc.dram_tensor("wq", [D, 512], bf16, kind="ExternalInput")
    wk = nc.dram_tensor("wk", [D, 512], bf16, kind="ExternalInput")
    wv = nc.dram_tensor("wv", [D, 512], bf16, kind="ExternalInput")
    wo = nc.dram_tensor("wo", [512, D], bf16, kind="ExternalInput")
    kct = nc.dram_tensor("kct", [128, CACHE], bf16, kind="ExternalInput")
    vc0 = nc.dram_tensor("vc0", [CACHE, DH], bf16, kind="ExternalInput")
    vc1 = nc.dram_tensor("vc1", [CACHE, DH], bf16, kind="ExternalInput")
    alibi = nc.dram_tensor("alibi", [128, 8, NCH], f32, kind="ExternalInput")
    bq = nc.dram_tensor("bq", [128, 4], f32, kind="ExternalInput")
    bo = nc.dram_tensor("bo", [128, 8], f32, kind="ExternalInput")
    bvb = nc.dram_tensor("bvb", [128, 512], f32, kind="ExternalInput")
    ones = nc.dram_tensor("ones", [128, DH], bf16, kind="ExternalInput")

    ko = nc.dram_tensor("ko", [512, Tq], bf16, kind="ExternalOutput")
    vo = nc.dram_tensor("vo", [Tq, 512], bf16, kind="ExternalOutput")
    oo = nc.dram_tensor("oo", [D, Tq], bf16, kind="ExternalOutput")

    with tile.TileContext(nc) as tc:
        with (
            tc.tile_pool(name="const", bufs=1) as cp,
            tc.tile_pool(name="pt", bufs=4) as ptp,
            tc.tile_pool(name="rc", bufs=2) as rcp,
            tc.tile_pool(name="mm", bufs=2, space="PSUM") as mmp,
            tc.tile_pool(name="otp", bufs=1, space="PSUM") as otp,
            tc.tile_pool(name="dnp", bufs=1, space="PSUM") as dnp,
        ):
            # resident SBUF tensors
            xt_sb = cp.tile([128, 8, Tq], bf16)
            wq_sb = cp.tile([128, 8, 512], bf16)
            wk_sb = cp.tile([128, 8, 512], bf16)
            wv_sb = cp.tile([128, 8, 512], bf16)
            wo_sb = cp.tile([128, 4, Tq], bf16)
            kct_sb = cp.tile([128, CACHE], bf16)
            vc_sb = [cp.tile([128, 8, DH], bf16, name=f"vc_sb{i}") for i in (0, 1)]
            alibi_sb = cp.tile([128, 8, NCH], f32)
            bq_sb = cp.tile([128, 4], f32)
            bo_sb = cp.tile([128, 8], f32)
            bvb_sb = cp.tile([128, 512], f32)
            ones_sb = cp.tile([128, DH], bf16)
            qt_sb = cp.tile([128, 4, Tq], bf16)
            kt_sb = cp.tile([128, 4, Tq], bf16)
            v_sb = cp.tile([128, 8, 512], bf16)
            wvt_sb = cp.tile([128, 4, Tq], bf16)
            ot_sb = cp.tile([128, 8, Tq], bf16)

            nc.sync.dma_start(out=xt_sb[:], in_=xt[:].rearrange("(c p) t -> p c t", p=128))
            nc.sync.dma_start(out=wq_sb[:], in_=wq[:].rearrange("(c p) n -> p c n", p=128))
            nc.sync.dma_start(out=wk_sb[:], in_=wk[:].rearrange("(c p) n -> p c n", p=128))
            nc.sync.dma_start(out=wv_sb[:], in_=wv[:].rearrange("(c p) n -> p c n", p=128))
            nc.sync.dma_start(out=wo_sb[:], in_=wo[:].rearrange("(c p) n -> p c n", p=128))
            nc.sync.dma_start(out=kct_sb[:], in_=kct[:])
            nc.sync.dma_start(out=vc_sb[0][:], in_=vc0[:].rearrange("(c p) d -> p c d", p=128))
            nc.sync.dma_start(out=vc_sb[1][:], in_=vc1[:].rearrange("(c p) d -> p c d", p=128))
            nc.sync.dma_start(out=alibi_sb[:], in_=alibi[:])
            nc.sync.dma_start(out=bq_sb[:], in_=bq[:])
            nc.sync.dma_start(out=bo_sb[:], in_=bo[:])
            nc.sync.dma_start(out=bvb_sb[:], in_=bvb[:])
            nc.sync.dma_start(out=ones_sb[:], in_=ones[:])

            # ---- phase 1: projections ----
            # kT_new [512, Tq] and qT [512, Tq]
            for m in range(4):
                ps = mmp.tile([128, 1024], f32, tag="mm")
                for qh in range(2):
                    for kc in range(8):
                        nc.tensor.matmul(
                            ps[:, qh * 512:(qh + 1) * 512],
                            lhsT=wk_sb[:, kc, m * 128:(m + 1) * 128],
                            rhs=xt_sb[:, kc, qh * 512:(qh + 1) * 512],
                            start=(kc == 0), stop=(kc == 7),
                        )
                nc.vector.tensor_copy(out=kt_sb[:, m, :], in_=ps[:])
            nc.sync.dma_start(out=ko[:].rearrange("(c p) t -> p c t", p=128), in_=kt_sb[:])

            # v_new [Tq, 512]
            for t8 in range(8):
                ps = mmp.tile([128, 1024], f32, tag="mm")
                for kc in range(8):
                    nc.tensor.matmul(
                        ps[:, :512],
                        lhsT=xt_sb[:, kc, t8 * 128:(t8 + 1) * 128],
                        rhs=wv_sb[:, kc, :],
                        start=(kc == 0), stop=(kc == 7),
                    )
                nc.vector.tensor_tensor(v_sb[:, t8, :], ps[:, :512], bvb_sb[:], ADD)
            nc.sync.dma_start(out=vo[:].rearrange("(c p) d -> p c d", p=128), in_=v_sb[:])

            for m in range(4):
                ps = mmp.tile([128, 1024], f32, tag="mm")
                for qh in range(2):
                    for kc in range(8):
                        nc.tensor.matmul(
                            ps[:, qh * 512:(qh + 1) * 512],
                            lhsT=wq_sb[:, kc, m * 128:(m + 1) * 128],
                            rhs=xt_sb[:, kc, qh * 512:(qh + 1) * 512],
                            start=(kc == 0), stop=(kc == 7),
                        )
                nc.vector.tensor_scalar(qt_sb[:, m, :], ps[:], bq_sb[:, m:m + 1], None, ADD)

            # ---- phase 2: attention (4 slot-pairs) ----
            for p in range(4):
                j0_pair = NCH - SLOT_W[2 * p] // 128
                ot_ps = otp.tile([128, 1024], f32, tag="ot")
                dn_ps = dnp.tile([128, 1024], f32, tag="dn")
                for j in range(j0_pair, NCH):
                    for d in (0, 1):
                        s = 2 * p + d
                        j0_s = NCH - SLOT_W[s] // 128
                        if j < j0_s:
                            continue
                        first = j == j0_s
                        last = j == NCH - 1
                        if j < 8:
                            klh = kct_sb[d * 64:(d + 1) * 64, j * 128:(j + 1) * 128]
                            vsrc = vc_sb[s][:, j, :]
                        else:
                            klh = kt_sb[d * 64:(d + 1) * 64, p, (j - 8) * 128:(j - 7) * 128]
                            vsrc = v_sb[:, j - 8, s * 64:(s + 1) * 64]
                        sc = mmp.tile([128, 1024], f32, tag="mm")
                        for qh in range(2):
                            nc.tensor.matmul(
                                sc[:, qh * 512:(qh + 1) * 512],
                                lhsT=klh,
                                rhs=qt_sb[d * 64:(d + 1) * 64, p, qh * 512:(qh + 1) * 512],
                                start=True, stop=True,
                            )
                        pt = ptp.tile([128, 1024], bf16, tag="pt")
                        nc.scalar.activation(pt[:], sc[:], EXP,
                                             bias=alibi_sb[:, s, j:j + 1], scale=0.125)
                        for qh in range(2):
                            nc.tensor.matmul(
                                ot_ps[d * 64:(d + 1) * 64, qh * 512:(qh + 1) * 512],
                                lhsT=vsrc,
                                rhs=pt[:, qh * 512:(qh + 1) * 512],
                                start=first, stop=last,
                                tile_position=(0, d * 64),
                                skip_group_check=True,
                            )
                            nc.tensor.matmul(
                                dn_ps[d * 64:(d + 1) * 64, qh * 512:(qh + 1) * 512],
                                lhsT=ones_sb[:],
                                rhs=pt[:, qh * 512:(qh + 1) * 512],
                                start=first, stop=last,
                                tile_position=(0, d * 64),
                                skip_group_check=True,
                            )
                rc = rcp.tile([128, 1024], f32, tag="rc")
                nc.vector.reciprocal_approx_fast(rc[:], dn_ps[:])
                nc.vector.tensor_tensor(wvt_sb[:, p, :], ot_ps[:], rc[:], MUL)

            # ---- phase 3: out projection ----
            for m in range(8):
                ps = mmp.tile([128, 1024], f32, tag="mm")
                for qh in range(2):
                    for kc in range(4):
                        nc.tensor.matmul(
                            ps[:, qh * 512:(qh + 1) * 512],
                            lhsT=wo_sb[:, kc, m * 128:(m + 1) * 128],
                            rhs=wvt_sb[:, kc, qh * 512:(qh + 1) * 512],
                            start=(kc == 0), stop=(kc == 3),
                        )
                nc.vector.tensor_scalar(ot_sb[:, m, :], ps[:], bo_sb[:, m:m + 1], None, ADD)
            nc.sync.dma_start(out=oo[:].rearrange("(c p) t -> p c t", p=128), in_=ot_sb[:])

    nc.compile()
    return nc


def _get_compiled():
    global _COMPILED
    if _COMPILED is None:
        _COMPILED = _build()
    return _COMPILED


def _reference_numpy(x, k_cache, v_cache, mask, Wq, bq, Wk, Wv, bv, Wo, bo):
    """Exact numpy fallback (used only if mask is nonzero)."""
    q = x @ Wq + bq
    k = np.concatenate([k_cache, x @ Wk], axis=1)
    v = np.concatenate([v_cache, x @ Wv + bv], axis=1)
    kn, vn = k[:, -CACHE:, :], v[:, -CACHE:, :]
    qh = q.reshape(B, Tq, H, DH).transpose(0, 2, 1, 3)
    kh = k.reshape(B, Tk, H, DH).transpose(0, 2, 1, 3)
    vh = v.reshape(B, Tk, H, DH).transpose(0, 2, 1, 3)
    slopes = 2.0 ** (-(8.0 / H) * np.arange(1, H + 1))
    rel = np.arange(Tk - 1, -1, -1, dtype=np.float32)
    bias = (-(slopes[:, None] * rel[None, :])).astype(np.float32)[None, :, None, :]
    scores = np.einsum("bhqd,bhkd->bhqk", qh, kh) / np.sqrt(DH) + mask + bias
    scores -= scores.max(axis=-1, keepdims=True)
    e = np.exp(scores)
    attn = e / e.sum(axis=-1, keepdims=True)
    a = np.einsum("bhqk,bhkd->bhqd", attn, vh)
    out = a.transpose(0, 2, 1, 3).reshape(B, Tq, D) @ Wo + bo
    return (out.astype(np.float32), kn.astype(np.float32), vn.astype(np.float32))


def _make_in_maps(inputs):
    x = np.asarray(inputs["x"], np.float32)
    k_cache = np.asarray(inputs["k_cache"], np.float32)
    v_cache = np.asarray(inputs["v_cache"], np.float32)
    Wq, bq = np.asarray(inputs["Wq"], np.float32), np.asarray(inputs["bq"], np.float32)
    Wk = np.asarray(inputs["Wk"], np.float32)
    Wv, bv = np.asarray(inputs["Wv"], np.float32), np.asarray(inputs["bv"], np.float32)
    Wo, bo = np.asarray(inputs["Wo"], np.float32), np.asarray(inputs["bo"], np.float32)

    alibi_g = []
    for g in (0, 1):
        heads = HEADS_OF_GROUP[g]
        al = np.empty((128, 8, NCH), np.float32)
        kpos = np.arange(128)
        for s in range(8):
            sl = _slopes[heads[s]]
            for j in range(NCH):
                al[:, s, j] = -sl * (Tk - 1 - (j * 128 + kpos))
        alibi_g.append(al)

    ones_arr = np.ones((128, DH), BF16)
    in_maps = []
    for c in range(8):
        b, g = c // 2, c % 2
        heads = HEADS_OF_GROUP[g]
        cols = COLS_G[g]
        kct_arr = np.concatenate(
            [k_cache[b][:, heads[s] * DH:(heads[s] + 1) * DH].T for s in (0, 1)], axis=0
        ).astype(BF16)
        in_maps.append({
            "xt": np.ascontiguousarray(x[b].T).astype(BF16),
            "wq": Wq[:, cols].astype(BF16),
            "wk": Wk[:, cols].astype(BF16),
            "wv": Wv[:, cols].astype(BF16),
            "wo": Wo[cols, :].astype(BF16),
            "kct": kct_arr,
            "vc0": v_cache[b][:, heads[0] * DH:(heads[0] + 1) * DH].astype(BF16),
            "vc1": v_cache[b][:, heads[1] * DH:(heads[1] + 1) * DH].astype(BF16),
            "alibi": alibi_g[g],
            "bq": np.ascontiguousarray(bq[cols].reshape(4, 128).T),
            "bo": np.ascontiguousarray((0.5 * bo).reshape(8, 128).T),
            "bvb": np.ascontiguousarray(np.broadcast_to(bv[cols], (128, 512))),
            "ones": ones_arr,
        })
    return in_maps


def kernel(x, k_cache, v_cache, mask, Wq, bq, Wk, Wv, bv, Wo, bo):
    mask = np.asarray(mask, np.float32)
    if np.any(mask):
        return _reference_numpy(
            np.asarray(x, np.float32), np.asarray(k_cache, np.float32),
            np.asarray(v_cache, np.float32), mask,
            np.asarray(Wq, np.float32), np.asarray(bq, np.float32),
            np.asarray(Wk, np.float32), np.asarray(Wv, np.float32),
            np.asarray(bv, np.float32), np.asarray(Wo, np.float32),
            np.asarray(bo, np.float32))

    from concourse.bass_utils import run_bass_kernel_spmd

    nc = _get_compiled()
    in_maps = _make_in_maps(dict(x=x, k_cache=k_cache, v_cache=v_cache, Wq=Wq,
                                 bq=bq, Wk=Wk, Wv=Wv, bv=bv, Wo=Wo, bo=bo))
    res = run_bass_kernel_spmd(nc, in_maps, core_ids=list(range(8))).results

    out = np.empty((B, Tq, D), np.float32)
    kn = np.empty((B, CACHE, D), np.float32)
    vn = np.empty((B, CACHE, D), np.float32)
    for b in range(B):
        acc = res[2 * b]["oo"].astype(np.float32) + res[2 * b + 1]["oo"].astype(np.float32)
        out[b] = acc.T
        for g in (0, 1):
            r = res[2 * b + g]
            kn[b][:, COLS_G[g]] = r["ko"].astype(np.float32).T
            vn[b][:, COLS_G[g]] = r["vo"].astype(np.float32)
    return out, kn, vn


# revision 10
# speedup vs baseline: 1.2713x; 1.2713x over previous
"""ALiBi multi-head attention (B=4, Tq=1024, D=1024, H=16, cache=1024) on 8
Trainium2 NeuronCores.

Sharding: core c = (batch b = c//2, head-group g = c%2). Each core runs one
batch with 8 of the 16 heads. Heads are assigned to groups interleaved by
ALiBi window size so per-core work balances, and all cores run an identical
SPMD graph (per-slot key windows are the max over the two groups).

Device dataflow (all transposed so ALiBi becomes a per-partition ACT bias):
  qT = Wq_s^T x^T + bq        kTnew = Wk_s^T x^T        vnew = x Wv_s + bv
  scoresT[k,q] = kT_chunk^T qT        (K=64 contraction, 2-head row packing)
  P^T = exp(0.125*scoresT + alibi[k])  (single ScalarE pass, PSUM->SBUF bf16)
  oT += v_chunk^T P^T  (V stationary, col-packed pairs -> PSUM[128,1024])
  denom += ones^T P^T  (M=64 ones lhsT -> denominator broadcast to 64 rows)
  wvT = oT * recip(denom);  outT_partial = Wo_s^T wvT + 0.5*bo
Softmax max-subtraction is skipped (scores are O(1), bias <= 0) and keys with
alibi bias < -16 are dropped (error ~1e-7 << tolerance).

Host: shards/pre-transposes inputs (bf16), sums the two partial outT per
batch, scatters head-sliced k/v outputs. No collectives on device.
"""

import numpy as np
import ml_dtypes

BF16 = ml_dtypes.bfloat16

B, Tq, D = 4, 1024, 1024
H, DH = 16, 64
CACHE = 1024
Tk = CACHE + Tq
NCH = Tk // 128  # 16 key chunks
T_CUT = 16.0

# ---- head assignment / windows (hardcoded, deterministic) ----
_slopes = 2.0 ** (-(8.0 / H) * np.arange(1, H + 1))  # head h=0..15 -> slope
_raw = np.minimum(np.ceil(T_CUT / _slopes), Tk)
_W = np.minimum(((_raw + 127) // 128) * 128, Tk).astype(int)
_order = np.argsort(-_W, kind="stable")
SLOT_W = [int(max(_W[_order[2 * s]], _W[_order[2 * s + 1]])) for s in range(8)]
HEADS_OF_GROUP = [[int(_order[2 * s + g]) for s in range(8)] for g in (0, 1)]
COLS_G = [
    np.concatenate([np.arange(h * DH, (h + 1) * DH) for h in HEADS_OF_GROUP[g]])
    for g in (0, 1)
]

_COMPILED = None


def _build():
    import concourse.bacc as bacc
    import concourse.tile as tile
    import concourse.mybir as mybir

    f32 = mybir.dt.float32
    bf16 = mybir.dt.bfloat16
    ADD = mybir.AluOpType.add
    MUL = mybir.AluOpType.mult
    EXP = mybir.ActivationFunctionType.Exp

    nc = bacc.Bacc("TRN2", target_bir_lowering=False, debug=False,
                   num_swdge_queues=4)

    xt = nc.dram_tensor("xt", [D, Tq], bf16, kind="ExternalInput")
    wq = nc.dram_tensor("wq", [D, 512], bf16, kind="ExternalInput")
    wk = nc.dram_tensor("wk", [D, 512], bf16, kind="ExternalInput")
    wv = nc.dram_tensor("wv", [D, 512], bf16, kind="ExternalInput")
    wo = nc.dram_tensor("wo", [512, D], bf16, kind="ExternalInput")
    kct = nc.dram_tensor("kct", [128, CACHE], bf16, kind="ExternalInput")
    vc0 = nc.dram_tensor("vc0", [CACHE, DH], bf16, kind="ExternalInput")
    vc1 = nc.dram_tensor("vc1", [CACHE, DH], bf16, kind="ExternalInput")
    alibi = nc.dram_tensor("alibi", [128, 8, NCH], f32, kind="ExternalInput")
    bq = nc.dram_tensor("bq", [128, 4], f32, kind="ExternalInput")
    bo = nc.dram_tensor("bo", [128, 8], f32, kind="ExternalInput")
    bvb = nc.dram_tensor("bvb", [128, 512], f32, kind="ExternalInput")

    ko = nc.dram_tensor("ko", [512, Tq], bf16, kind="ExternalOutput")
    vo = nc.dram_tensor("vo", [Tq, 512], bf16, kind="ExternalOutput")
    oo = nc.dram_tensor("oo", [D, Tq], bf16, kind="ExternalOutput")

    ko_r = ko[:].rearrange("(c p) t -> p c t", p=128)
    vo_r = vo[:].rearrange("(c p) (s e) -> p c s e", p=128, s=8)
    oo_r = oo[:].rearrange("(c p) t -> p c t", p=128)

    with tile.TileContext(nc) as tc:
        with (
            tc.tile_pool(name="const", bufs=1) as cp,
            tc.tile_pool(name="pt", bufs=4) as ptp,
            tc.tile_pool(name="rc", bufs=2) as rcp,
            tc.tile_pool(name="bc", bufs=2) as bcp,
            tc.tile_pool(name="sc", bufs=2, space="PSUM") as scp,
            tc.tile_pool(name="proj", bufs=1, space="PSUM") as projp,
            tc.tile_pool(name="otp", bufs=1, space="PSUM") as otp,
        ):
            # resident SBUF tensors
            xt_sb = cp.tile([128, 8, Tq], bf16)
            wq_sb = cp.tile([128, 8, 512], bf16)
            wk_sb = cp.tile([128, 8, 512], bf16)
            wv_sb = cp.tile([128, 8, 512], bf16)
            wo_sb = cp.tile([128, 4, Tq], bf16)
            kct_sb = cp.tile([128, CACHE], bf16)
            vc_sb = [cp.tile([128, 8, DH + 1], bf16, name=f"vc_sb{i}") for i in (0, 1)]
            alibi_sb = cp.tile([128, 8, NCH], f32)
            bq_sb = cp.tile([128, 4], f32)
            bo_sb = cp.tile([128, 8], f32)
            bvb_sb = cp.tile([128, 512], f32)
            qt_sb = cp.tile([128, 4, Tq], bf16)
            kt_sb = cp.tile([128, 4, Tq], bf16)
            v_sb = cp.tile([128, 8, 8, DH + 1], bf16)
            wvt_sb = cp.tile([128, 4, Tq], bf16)
            ot_sb = cp.tile([128, 8, Tq], bf16)

            # loads in order of first use
            nc.sync.dma_start(out=xt_sb[:], in_=xt[:].rearrange("(c p) t -> p c t", p=128))
            nc.sync.dma_start(out=wq_sb[:], in_=wq[:].rearrange("(c p) n -> p c n", p=128))
            nc.sync.dma_start(out=bq_sb[:], in_=bq[:])
            nc.sync.dma_start(out=kct_sb[:], in_=kct[:])
            nc.sync.dma_start(out=vc_sb[0][:, :, 0:DH], in_=vc0[:].rearrange("(c p) d -> p c d", p=128))
            nc.sync.dma_start(out=vc_sb[1][:, :, 0:DH], in_=vc1[:].rearrange("(c p) d -> p c d", p=128))
            nc.sync.dma_start(out=alibi_sb[:], in_=alibi[:])
            nc.sync.dma_start(out=wk_sb[:], in_=wk[:].rearrange("(c p) n -> p c n", p=128))
            nc.sync.dma_start(out=wv_sb[:], in_=wv[:].rearrange("(c p) n -> p c n", p=128))
            nc.sync.dma_start(out=bvb_sb[:], in_=bvb[:])
            nc.sync.dma_start(out=wo_sb[:], in_=wo[:].rearrange("(c p) n -> p c n", p=128))
            nc.sync.dma_start(out=bo_sb[:], in_=bo[:])
            # ones columns for the augmented V (softmax denominators)
            nc.vector.memset(v_sb[:, :, :, DH:DH + 1], 1.0)
            nc.vector.memset(vc_sb[0][:, :, DH:DH + 1], 1.0)
            nc.vector.memset(vc_sb[1][:, :, DH:DH + 1], 1.0)

            # ---- qT projection (uses "sc" psum tag; attention not started) ----
            for m in range(4):
                ps = scp.tile([128, 1024], f32, tag="sc")
                for qh in range(2):
                    for kc in range(8):
                        nc.tensor.matmul(
                            ps[:, qh * 512:(qh + 1) * 512],
                            lhsT=wq_sb[:, kc, m * 128:(m + 1) * 128],
                            rhs=xt_sb[:, kc, qh * 512:(qh + 1) * 512],
                            start=(kc == 0), stop=(kc == 7),
                        )
                nc.vector.tensor_scalar(qt_sb[:, m, :], ps[:], bq_sb[:, m:m + 1], None, ADD)

            # ---- filler generators: kT / v projections, pumped during attention ----
            def kt_gen(m):
                ps = projp.tile([128, 1024], f32, tag="proj")
                for qh in range(2):
                    for kc in range(8):
                        nc.tensor.matmul(
                            ps[:, qh * 512:(qh + 1) * 512],
                            lhsT=wk_sb[:, kc, m * 128:(m + 1) * 128],
                            rhs=xt_sb[:, kc, qh * 512:(qh + 1) * 512],
                            start=(kc == 0), stop=(kc == 7),
                        )
                        yield
                nc.vector.tensor_copy(out=kt_sb[:, m, :], in_=ps[:])
                nc.sync.dma_start(out=ko_r[:, m, :], in_=kt_sb[:, m, :])
                yield

            def v_gen(t8):
                ps = projp.tile([128, 1024], f32, tag="proj")
                for kc in range(8):
                    nc.tensor.matmul(
                        ps[:, :512],
                        lhsT=xt_sb[:, kc, t8 * 128:(t8 + 1) * 128],
                        rhs=wv_sb[:, kc, :],
                        start=(kc == 0), stop=(kc == 7),
                    )
                    yield
                nc.vector.tensor_tensor(
                    v_sb[:, t8, :, 0:DH],
                    ps[:, :512].rearrange("p (s e) -> p s e", s=8),
                    bvb_sb[:].rearrange("p (s e) -> p s e", s=8), ADD)
                nc.sync.dma_start(out=vo_r[:, t8, :, :], in_=v_sb[:, t8, :, 0:DH])
                yield

            filler = [kt_gen(0)] + [v_gen(t) for t in range(8)] + \
                     [kt_gen(1), kt_gen(2), kt_gen(3)]

            def pump(n):
                while n > 0 and filler:
                    try:
                        next(filler[0])
                        n -= 1
                    except StopIteration:
                        filler.pop(0)

            # ---- attention: slot-major, AV lags exp by 2 items ----
            for p in range(4):
                for d in (0, 1):
                    s = 2 * p + d
                    W = SLOT_W[s]
                    j_first = NCH - W // 128
                    ot = otp.tile([65, 1024], f32, tag="ot")

                    def av_emit(j, pt, ot=ot, s=s, p=p, d=d, j_first=j_first):
                        if j < 8:
                            vsrc = vc_sb[s][:, j, :]
                        else:
                            vsrc = v_sb[:, j - 8, s, :]
                        for qh in range(2):
                            nc.tensor.matmul(
                                ot[:, qh * 512:(qh + 1) * 512],
                                lhsT=vsrc,
                                rhs=pt[:, qh * 512:(qh + 1) * 512],
                                start=(j == j_first), stop=(j == NCH - 1),
                                skip_group_check=True,
                            )

                    pend = []
                    for j in range(j_first, NCH):
                        sc = scp.tile([128, 1024], f32, tag="sc")
                        if j < 8:
                            klh = kct_sb[d * 64:(d + 1) * 64, j * 128:(j + 1) * 128]
                        else:
                            klh = kt_sb[d * 64:(d + 1) * 64, p, (j - 8) * 128:(j - 7) * 128]
                        for qh in range(2):
                            nc.tensor.matmul(
                                sc[:, qh * 512:(qh + 1) * 512],
                                lhsT=klh,
                                rhs=qt_sb[d * 64:(d + 1) * 64, p, qh * 512:(qh + 1) * 512],
                                start=True, stop=True,
                            )
                        pt = ptp.tile([128, 1024], bf16, tag="pt")
                        nc.scalar.activation(pt[:], sc[:], EXP,
                                             bias=alibi_sb[:, s, j:j + 1], scale=0.125)
                        pump(6)
                        pend.append((j, pt))
                        if len(pend) > 2:
                            av_emit(*pend.pop(0))
                    for it in pend:
                        av_emit(*it)

                    # normalize: wvT rows = oT * recip(denom row)
                    rc = rcp.tile([1, 1024], f32, tag="rc")
                    nc.vector.reciprocal_approx_fast(rc[:], ot[64:65, :])
                    bc = bcp.tile([64, 1024], f32, tag="bc")
                    nc.gpsimd.partition_broadcast(bc[:], rc[:])
                    nc.vector.tensor_tensor(
                        wvt_sb[d * 64:(d + 1) * 64, p, :], ot[0:64, :], bc[:], MUL)

            # ---- out projection ----
            for m in range(8):
                ps = scp.tile([128, 1024], f32, tag="sc")
                for qh in range(2):
                    for kc in range(4):
                        nc.tensor.matmul(
                            ps[:, qh * 512:(qh + 1) * 512],
                            lhsT=wo_sb[:, kc, m * 128:(m + 1) * 128],
                            rhs=wvt_sb[:, kc, qh * 512:(qh + 1) * 512],
                            start=(kc == 0), stop=(kc == 3),
                        )
                nc.vector.tensor_scalar(ot_sb[:, m, :], ps[:], bo_sb[:, m:m + 1], None, ADD)
                nc.sync.dma_start(out=oo_r[:, m, :], in_=ot_sb[:, m, :])

    nc.compile()
    return nc


def _get_compiled():
    global _COMPILED
    if _COMPILED is None:
        _COMPILED = _build()
    return _COMPILED


def _reference_numpy(x, k_cache, v_cache, mask, Wq, bq, Wk, Wv, bv, Wo, bo):
    """Exact numpy fallback (used only if mask is nonzero)."""
    q = x @ Wq + bq
    k = np.concatenate([k_cache, x @ Wk], axis=1)
    v = np.concatenate([v_cache, x @ Wv + bv], axis=1)
    kn, vn = k[:, -CACHE:, :], v[:, -CACHE:, :]
    qh = q.reshape(B, Tq, H, DH).transpose(0, 2, 1, 3)
    kh = k.reshape(B, Tk, H, DH).transpose(0, 2, 1, 3)
    vh = v.reshape(B, Tk, H, DH).transpose(0, 2, 1, 3)
    slopes = 2.0 ** (-(8.0 / H) * np.arange(1, H + 1))
    rel = np.arange(Tk - 1, -1, -1, dtype=np.float32)
    bias = (-(slopes[:, None] * rel[None, :])).astype(np.float32)[None, :, None, :]
    scores = np.einsum("bhqd,bhkd->bhqk", qh, kh) / np.sqrt(DH) + mask + bias
    scores -= scores.max(axis=-1, keepdims=True)
    e = np.exp(scores)
    attn = e / e.sum(axis=-1, keepdims=True)
    a = np.einsum("bhqk,bhkd->bhqd", attn, vh)
    out = a.transpose(0, 2, 1, 3).reshape(B, Tq, D) @ Wo + bo
    return (out.astype(np.float32), kn.astype(np.float32), vn.astype(np.float32))


def _make_in_maps(inputs):
    x = np.asarray(inputs["x"], np.float32)
    k_cache = np.asarray(inputs["k_cache"], np.float32)
    v_cache = np.asarray(inputs["v_cache"], np.float32)
    Wq, bq = np.asarray(inputs["Wq"], np.float32), np.asarray(inputs["bq"], np.float32)
    Wk = np.asarray(inputs["Wk"], np.float32)
    Wv, bv = np.asarray(inputs["Wv"], np.float32), np.asarray(inputs["bv"], np.float32)
    Wo, bo = np.asarray(inputs["Wo"], np.float32), np.asarray(inputs["bo"], np.float32)

    alibi_g = []
    for g in (0, 1):
        heads = HEADS_OF_GROUP[g]
        al = np.empty((128, 8, NCH), np.float32)
        kpos = np.arange(128)
        for s in range(8):
            sl = _slopes[heads[s]]
            for j in range(NCH):
                al[:, s, j] = -sl * (Tk - 1 - (j * 128 + kpos))
        alibi_g.append(al)

    in_maps = []
    for c in range(8):
        b, g = c // 2, c % 2
        heads = HEADS_OF_GROUP[g]
        cols = COLS_G[g]
        kct_arr = np.concatenate(
            [k_cache[b][:, heads[s] * DH:(heads[s] + 1) * DH].T for s in (0, 1)], axis=0
        ).astype(BF16)
        in_maps.append({
            "xt": np.ascontiguousarray(x[b].T).astype(BF16),
            "wq": Wq[:, cols].astype(BF16),
            "wk": Wk[:, cols].astype(BF16),
            "wv": Wv[:, cols].astype(BF16),
            "wo": Wo[cols, :].astype(BF16),
            "kct": kct_arr,
            "vc0": v_cache[b][:, heads[0] * DH:(heads[0] + 1) * DH].astype(BF16),
            "vc1": v_cache[b][:, heads[1] * DH:(heads[1] + 1) * DH].astype(BF16),
            "alibi": alibi_g[g],
            "bq": np.ascontiguousarray(bq[cols].reshape(4, 128).T),
            "bo": np.ascontiguousarray((0.5 * bo).reshape(8, 128).T),
            "bvb": np.ascontiguousarray(np.broadcast_to(bv[cols], (128, 512))),
        })
    return in_maps


def kernel(x, k_cache, v_cache, mask, Wq, bq, Wk, Wv, bv, Wo, bo):
    mask = np.asarray(mask, np.float32)
    if np.any(mask):
        return _reference_numpy(
            np.asarray(x, np.float32), np.asarray(k_cache, np.float32),
            np.asarray(v_cache, np.float32), mask,
            np.asarray(Wq, np.float32), np.asarray(bq, np.float32),
            np.asarray(Wk, np.float32), np.asarray(Wv, np.float32),
            np.asarray(bv, np.float32), np.asarray(Wo, np.float32),
            np.asarray(bo, np.float32))

    from concourse.bass_utils import run_bass_kernel_spmd

    nc = _get_compiled()
    in_maps = _make_in_maps(dict(x=x, k_cache=k_cache, v_cache=v_cache, Wq=Wq,
                                 bq=bq, Wk=Wk, Wv=Wv, bv=bv, Wo=Wo, bo=bo))
    res = run_bass_kernel_spmd(nc, in_maps, core_ids=list(range(8))).results

    out = np.empty((B, Tq, D), np.float32)
    kn = np.empty((B, CACHE, D), np.float32)
    vn = np.empty((B, CACHE, D), np.float32)
    for b in range(B):
        acc = res[2 * b]["oo"].astype(np.float32) + res[2 * b + 1]["oo"].astype(np.float32)
        out[b] = acc.T
        for g in (0, 1):
            r = res[2 * b + g]
            kn[b][:, COLS_G[g]] = r["ko"].astype(np.float32).T
            vn[b][:, COLS_G[g]] = r["vo"].astype(np.float32)
    return out, kn, vn
